# revision 8
# baseline (speedup 1.0000x reference)
"""Two-layer GCN (PyG GCNConv semantics) on 8 Trainium2 NeuronCores.

Strategy (sharding_hint): nodes are sharded row-wise across the 8 cores;
edges are partitioned by destination node so the segment-sum stays local;
source-node features are exchanged with an on-device AllGather between
layers; the small weight matrices are replicated.

Gather pipeline: per-edge source rows are fetched with batched SWDGE
dma_gather (int16 indices, thousands of rows per instruction) instead of
per-128-row indirect DMAs — this takes the Pool-engine descriptor
generation off the critical path.  Because dma_gather indices are int16,
the gather table is processed in 4 ranges of 32768 rows; edges are grouped
host-side by (destination window, source range) and padded to 128-slot
tiles so every tile is single-window and single-range.

Aggregation per 128-node destination window: a norm-weighted one-hot
S[e, j] = norm[e] * (dst_rel[e] == j) built in one DVE tensor_scalar per
tile, then PE matmuls accumulate msgs into PSUM.  Epilogues: relu+bias
into an SBUF-resident fp16 transposed activation accumulator (layer 1);
bias + log_softmax (layer 2).

Edge bookkeeping (sorting, slot assignment, padding so all 8 cores share
one instruction stream) is host-side numpy index work; all floating-point
math on features runs on device.
"""

import math

import numpy as np

import concourse.bass as bass
import concourse.mybir as mybir
import concourse.tile as tile
from concourse import library_config
from concourse.bass_utils import run_bass_kernel_spmd

N_NODES = 100000
N_EDGES = 1600000
IN_DIM, HID_DIM, OUT_DIM = 128, 64, 40
N_CORES = 8

RNG = 25000          # rows per gather range (must fit int16 indices)
NR = 4               # number of ranges covering N_NODES
CHUNK = 32           # tiles per dma_gather instruction
GBUFS = 10           # gather chunk buffers in flight
SBUFS = 8
PBUFS = 4

F32 = mybir.dt.float32
F16 = mybir.dt.float16
I16 = mybir.dt.int16


def _split_long_waits(nc, max_waits=1):
    """This toolchain's codegen rejects instructions carrying more than one
    semaphore wait; move extra waits onto preceding same-engine no-ops."""
    cnt = 0
    for bb in nc.main_func.blocks:
        i = 0
        insts = bb.instructions
        while i < len(insts):
            ins = insts[i]
            si = ins.sync_info
            if si is not None and si.on_wait and len(si.on_wait) > max_waits:
                waits = list(si.on_wait)
                keep = waits[-max_waits:]
                extra = waits[:-max_waits]
                si.on_wait = keep
                new_insts = []
                for j in range(0, len(extra), max_waits):
                    chunk = extra[j : j + max_waits]
                    nop = mybir.InstNoOp(
                        name=f"{ins.name}-waitsplit-{j}",
                        engine=ins.engine,
                        ins=[],
                        outs=[],
                        sync_info=mybir.SyncInfo(on_wait=chunk, on_update=[]),
                    )
                    new_insts.append(nop)
                insts[i:i] = new_insts
                i += len(new_insts)
                cnt += len(new_insts)
            i += 1
    return cnt


def _preprocess(edge_index, n_nodes, n_cores):
    """Host-side index bookkeeping. Returns per-core slot arrays + layout."""
    nloc = n_nodes // n_cores
    wn = math.ceil(nloc / 128)

    src = np.asarray(edge_index[0], dtype=np.int64)
    dst = np.asarray(edge_index[1], dtype=np.int64)
    loop = np.arange(n_nodes, dtype=np.int64)
    src_all = np.concatenate([src, loop])
    dst_all = np.concatenate([dst, loop])

    deg = np.bincount(dst_all, minlength=n_nodes).astype(np.float64)
    dis = np.where(deg > 0, 1.0 / np.sqrt(deg), 0.0)
    norm = (dis[src_all] * dis[dst_all]).astype(np.float32)
    norm16 = norm.astype(np.float16)

    core = dst_all // nloc
    dloc = dst_all - core * nloc
    w = dloc >> 7
    drel = (dloc & 127).astype(np.float16)
    r = src_all // RNG
    assert r.max() < NR

    key = (core * wn + w) * NR + r
    order = np.argsort(key, kind="stable")
    counts = np.bincount(key, minlength=n_cores * wn * NR).reshape(n_cores, wn, NR)

    # tiles per (window, range): shared across cores (single SPMD program)
    k_wr = np.maximum((counts + 127) // 128, 0).max(axis=0)  # [wn, NR]
    k_wr[:, 0] = np.maximum(k_wr[:, 0], 1)  # every window needs >=1 tile

    T_r = k_wr.sum(axis=0)  # tiles per range  [NR]
    t_start = np.zeros(NR + 1, dtype=np.int64)
    t_start[1:] = np.cumsum(T_r)
    t_total = int(t_start[NR])

    # global tile id of the first tile of (w, r): r-major layout
    trw = np.zeros((NR, wn), dtype=np.int64)
    for rr in range(NR):
        trw[rr, 0] = t_start[rr]
        trw[rr, 1:] = t_start[rr] + np.cumsum(k_wr[:, rr])[:-1]

    # rank of each edge within its (core, w, r) group, in sorted order
    grp_start = np.zeros(n_cores * wn * NR, dtype=np.int64)
    grp_start[1:] = np.cumsum(counts.reshape(-1))[:-1]
    rank = np.arange(len(order), dtype=np.int64) - grp_start[key[order]]

    w_o = w[order]
    r_o = r[order]
    slot = (trw[r_o, w_o] + (rank >> 7)) * 128 + (rank & 127)

    n_slots = t_total * 128
    idx_a = np.zeros((n_cores, n_slots), dtype=np.int16)
    drel_a = np.zeros((n_cores, n_slots), dtype=np.float16)
    nrm_a = np.zeros((n_cores, n_slots), dtype=np.float16)

    c_o = core[order]
    idx_a[c_o, slot] = (src_all[order] - r_o * RNG).astype(np.int16)
    drel_a[c_o, slot] = drel[order]
    nrm_a[c_o, slot] = norm16[order]

    # device layouts:
    #   idx: [128, t_total*8] int16, slot s at [16g + s%16, s//16], g=0..7
    #   drel/nrm: [128, t_total] f32, slot s at [s%128, s//128]
    idx_dev, drel_dev, nrm_dev = [], [], []
    for c in range(n_cores):
        base = idx_a[c].reshape(n_slots // 16, 16).T  # [16, t_total*8]
        idx_dev.append(np.tile(base, (8, 1)).copy())
        drel_dev.append(drel_a[c].reshape(t_total, 128).T.copy())
        nrm_dev.append(nrm_a[c].reshape(t_total, 128).T.copy())

    return {
        "nloc": nloc,
        "wn": wn,
        "k_wr": k_wr,
        "T_r": [int(v) for v in T_r],
        "t_start": [int(v) for v in t_start],
        "trw": trw,
        "t_total": t_total,
        "idx": idx_dev,
        "drel": drel_dev,
        "norm": nrm_dev,
    }


def _build_nc(meta, n_nodes, hid, out_dim, in_dim, n_cores):
    nloc = meta["nloc"]
    wn = meta["wn"]
    k_wr = meta["k_wr"]
    T_r = meta["T_r"]
    t_start = meta["t_start"]
    trw = meta["trw"]
    t_total = meta["t_total"]

    nc = bass.Bass(num_devices=n_cores)

    xT16 = nc.dram_tensor("xT16", [in_dim, nloc], F16, kind="ExternalInput")
    idx = nc.dram_tensor("idx", [128, t_total * 8], I16, kind="ExternalInput")
    drel = nc.dram_tensor("drel", [128, t_total], F16, kind="ExternalInput")
    nrm = nc.dram_tensor("nrm", [128, t_total], F16, kind="ExternalInput")
    w1 = nc.dram_tensor("w1", [in_dim, hid], F16, kind="ExternalInput")
    w2p = nc.dram_tensor("w2p", [hid, hid], F16, kind="ExternalInput")
    b1c = nc.dram_tensor("b1c", [hid, 1], F32, kind="ExternalInput")
    b2rep = nc.dram_tensor("b2rep", [128, hid], F32, kind="ExternalInput")
    iota_in = nc.dram_tensor("iota", [128, 128], F16, kind="ExternalInput")
    out = nc.dram_tensor("out", [nloc, out_dim], F32, kind="ExternalOutput")

    nb = math.ceil(nloc / 128)
    eq = mybir.AluOpType.is_equal
    mul = mybir.AluOpType.mult

    with tile.TileContext(nc) as tc:
        with (
            tc.tile_pool(name="const", bufs=1) as cp,
            tc.tile_pool(name="gpool", bufs=GBUFS) as gp,
            tc.tile_pool(name="spool", bufs=SBUFS) as sp,
            tc.tile_pool(name="evac", bufs=4) as ep,
            tc.tile_pool(name="ps_agg", bufs=PBUFS, space="PSUM") as pa,
            tc.tile_pool(name="ps_mm", bufs=4, space="PSUM") as pm,
            tc.tile_pool(name="dram", bufs=1, space="DRAM") as dp,
        ):
            nc.gpsimd.load_library(library_config.mlp)

            # ---- resident tensors ----
            xT_t = cp.tile([in_dim, nloc], F16)
            nc.sync.dma_start(out=xT_t[:], in_=xT16[:])
            idx_t = cp.tile([128, t_total * 8], I16)
            nc.sync.dma_start(out=idx_t[:], in_=idx[:])
            drel_t = cp.tile([128, t_total], F16)
            nc.sync.dma_start(out=drel_t[:], in_=drel[:])
            nrm_t = cp.tile([128, t_total], F16)
            nc.sync.dma_start(out=nrm_t[:], in_=nrm[:])
            w1_t = cp.tile([in_dim, hid], F16)
            nc.sync.dma_start(out=w1_t[:], in_=w1[:])
            w2_t = cp.tile([hid, hid], F16)
            nc.sync.dma_start(out=w2_t[:], in_=w2p[:])
            b1_t = cp.tile([hid, 1], F32)
            nc.sync.dma_start(out=b1_t[:], in_=b1c[:])
            b2_t = cp.tile([128, hid], F32)
            nc.sync.dma_start(out=b2_t[:], in_=b2rep[:])
            iota_t = cp.tile([128, 128], F16)
            nc.sync.dma_start(out=iota_t[:], in_=iota_in[:])
            acc1T = cp.tile([hid, wn * 128], F16)

            h1loc = dp.tile([nloc, 2 * hid], F16)
            h1full = dp.tile([n_nodes, 2 * hid], F16, addr_space="Shared")
            h2loc = dp.tile([nloc, 2 * hid], F16)
            h2full = dp.tile([n_nodes, 2 * hid], F16, addr_space="Shared")

            def build_s(t):
                """S[e, j] = norm[e] * (dst_rel[e] == j), exact in f32."""
                s = sp.tile([128, 128], F16, tag="s", name="s")
                nc.vector.tensor_scalar(
                    out=s[:], in0=iota_t[:],
                    scalar1=drel_t[:, t : t + 1],
                    scalar2=nrm_t[:, t : t + 1],
                    op0=eq, op1=mul,
                )
                return s

            def pre_matmul(lhsT_t, w_t, dst_dram):
                """h = act_prev @ W for the local shard -> DRAM table."""
                for b in range(nb):
                    cols = min(128, nloc - b * 128)
                    ps = pm.tile([128, hid], F32, tag="pmm")
                    nc.tensor.matmul(
                        out=ps[:cols, :],
                        lhsT=lhsT_t[:, b * 128 : b * 128 + cols],
                        rhs=w_t[:],
                        start=True,
                        stop=True,
                    )
                    hb = ep.tile([128, hid], F16, tag="hb")
                    nc.vector.tensor_copy(out=hb[:cols, :], in_=ps[:cols, :])
                    nc.sync.dma_start(
                        out=dst_dram[b * 128 : b * 128 + cols, 0:hid],
                        in_=hb[:cols, :],
                    )

            def all_gather(src_dram, dst_dram):
                nc.gpsimd.collective_compute(
                    "AllGather",
                    mybir.AluOpType.bypass,
                    replica_groups=[list(range(n_cores))],
                    ins=[src_dram[:].opt()],
                    outs=[dst_dram[0:n_nodes, :].opt()],
                )

            nidx_regs = {}

            def nidx_reg(n):
                if n not in nidx_regs:
                    nidx_regs[n] = nc.gpsimd.to_reg(n)
                return nidx_regs[n]

            def agg_pass(table, layer):
                """Chunk-pipelined gather + per-window aggregation."""
                issued = [0] * NR          # tiles issued per range
                chunks = [[] for _ in range(NR)]  # (tile_ref, t0, span)

                def ensure(rr, tiles_needed):
                    while issued[rr] < tiles_needed:
                        t0 = t_start[rr] + issued[rr]
                        span = min(CHUNK, T_r[rr] - issued[rr])
                        g = gp.tile([128, CHUNK, 2 * hid], F16, tag="g", name="g")
                        rows0 = rr * RNG
                        rows1 = min(rows0 + RNG, n_nodes)
                        nc.gpsimd.dma_gather(
                            g[:, 0:span, :],
                            table[rows0:rows1, :],
                            idx_t[:, t0 * 8 : (t0 + span) * 8],
                            span * 128,
                            nidx_reg(span * 128),
                            2 * hid,
                            single_packet=False,
                        )
                        chunks[rr].append((g, t0, span))
                        issued[rr] += span

                def gview(rr, t):
                    while True:
                        g, t0, span = chunks[rr][0]
                        if t < t0 + span:
                            return g[:, t - t0, 0:hid]
                        chunks[rr].pop(0)

                for w in range(wn):
                    kw = [int(k_wr[w][rr]) for rr in range(NR)]
                    ktot = sum(kw)
                    for rr in range(NR):
                        if kw[rr]:
                            ensure(rr, trw[rr][w] - t_start[rr] + kw[rr])
                    if layer == 1:
                        pw = pa.tile([hid, 128], F32, tag="pagg")
                    else:
                        pw = pa.tile([128, hid], F32, tag="pagg")
                    ki = 0
                    for rr in range(NR):
                        for k in range(kw[rr]):
                            t = int(trw[rr][w]) + k
                            gv = gview(rr, t)
                            s = build_s(t)
                            if layer == 1:
                                nc.tensor.matmul(
                                    out=pw[:], lhsT=gv, rhs=s[:],
                                    start=(ki == 0), stop=(ki == ktot - 1),
                                )
                            else:
                                nc.tensor.matmul(
                                    out=pw[:], lhsT=s[:], rhs=gv,
                                    start=(ki == 0), stop=(ki == ktot - 1),
                                )
                            ki += 1
                    if layer == 1:
                        # epilogue: acc1T[:, w*128:] = relu(pw + b1)  (fp16)
                        nc.scalar.activation(
                            out=acc1T[:, w * 128 : (w + 1) * 128],
                            in_=pw[:],
                            func=mybir.ActivationFunctionType.Relu,
                            bias=b1_t[:],
                        )
                    else:
                        rows = min(128, nloc - w * 128)
                        zt = ep.tile([128, hid], F32, tag="zt")
                        nc.vector.tensor_tensor(
                            out=zt[:], in0=pw[:], in1=b2_t[:], op=mybir.AluOpType.add
                        )
                        mx = ep.tile([128, 1], F32, tag="mx")
                        nc.vector.reduce_max(
                            mx[:], zt[:, :out_dim], axis=mybir.AxisListType.X
                        )
                        sh = ep.tile([128, out_dim], F32, tag="sh")
                        nc.vector.tensor_scalar_sub(
                            out=sh[:], in0=zt[:, :out_dim], scalar1=mx[:]
                        )
                        ex = ep.tile([128, out_dim], F32, tag="ex")
                        sm = ep.tile([128, 1], F32, tag="sm")
                        nc.scalar.activation(
                            out=ex[:], in_=sh[:],
                            func=mybir.ActivationFunctionType.Exp,
                            accum_out=sm[:],
                        )
                        lnt = ep.tile([128, 1], F32, tag="lnt")
                        nc.scalar.activation(
                            out=lnt[:], in_=sm[:],
                            func=mybir.ActivationFunctionType.Ln,
                        )
                        res = ep.tile([128, out_dim], F32, tag="res")
                        nc.vector.tensor_scalar_sub(
                            out=res[:], in0=sh[:], scalar1=lnt[:]
                        )
                        nc.sync.dma_start(
                            out=out[w * 128 : w * 128 + rows, :], in_=res[:rows, :]
                        )

            # ---- pipeline ----
            pre_matmul(xT_t, w1_t, h1loc)
            all_gather(h1loc, h1full)
            agg_pass(h1full, layer=1)
            pre_matmul(acc1T, w2_t, h2loc)
            all_gather(h2loc, h2full)
            agg_pass(h2full, layer=2)

    _split_long_waits(nc)
    mybir.codegen_inst_isa_subclasses(nc)
    return nc


def _prepare(x, edge_index, W1, b1, W2, b2, n_nodes=N_NODES, n_cores=N_CORES):
    x = np.asarray(x, dtype=np.float32)
    W1 = np.asarray(W1, dtype=np.float32)
    b1 = np.asarray(b1, dtype=np.float32)
    W2 = np.asarray(W2, dtype=np.float32)
    b2 = np.asarray(b2, dtype=np.float32)

    in_dim = x.shape[1]
    hid = W1.shape[1]
    out_dim = W2.shape[1]

    meta = _preprocess(edge_index, n_nodes, n_cores)
    nloc = meta["nloc"]

    nc = _build_nc(meta, n_nodes, hid, out_dim, in_dim, n_cores)

    w2pad = np.zeros((hid, hid), dtype=np.float16)
    w2pad[:, :out_dim] = W2.astype(np.float16)
    b2p = np.zeros((hid,), dtype=np.float32)
    b2p[:out_dim] = b2
    b2rep = np.tile(b2p[None, :], (128, 1)).copy()
    b1c = b1.reshape(hid, 1).copy()
    iota = np.tile(np.arange(128, dtype=np.float16)[None, :], (128, 1)).copy()
    w1h = W1.astype(np.float16)

    in_maps = []
    for c in range(n_cores):
        xs = x[c * nloc : (c + 1) * nloc]
        in_maps.append(
            {
                "xT16": np.ascontiguousarray(xs.T.astype(np.float16)),
                "idx": meta["idx"][c],
                "drel": meta["drel"][c],
                "nrm": meta["norm"][c],
                "w1": w1h,
                "w2p": w2pad,
                "b1c": b1c,
                "b2rep": b2rep,
                "iota": iota,
            }
        )
    return nc, in_maps


def kernel(x, edge_index, W1, b1, W2, b2):
    nc, in_maps = _prepare(x, edge_index, W1, b1, W2, b2)
    res = run_bass_kernel_spmd(nc, in_maps, core_ids=list(range(N_CORES)))
    return np.concatenate([res.results[c]["out"] for c in range(N_CORES)], axis=0)


# revision 9
# speedup vs baseline: 1.7181x; 1.7181x over previous
"""Two-layer GCN (PyG GCNConv semantics) on 8 Trainium2 NeuronCores.

Strategy (sharding_hint): nodes are sharded row-wise across the 8 cores;
edges are partitioned by destination node so the segment-sum stays local;
source-node features are exchanged with an on-device AllGather between
layers; the small weight matrices are replicated.

Gather pipeline: per-edge source rows are fetched with batched SWDGE
dma_gather (int16 indices, thousands of rows per instruction) instead of
per-128-row indirect DMAs — this takes the Pool-engine descriptor
generation off the critical path.  Because dma_gather indices are int16,
the gather table is processed in 4 ranges of 32768 rows; edges are grouped
host-side by (destination window, source range) and padded to 128-slot
tiles so every tile is single-window and single-range.

Aggregation per 128-node destination window: a norm-weighted one-hot
S[e, j] = norm[e] * (dst_rel[e] == j) built in one DVE tensor_scalar per
tile, then PE matmuls accumulate msgs into PSUM.  Epilogues: relu+bias
into an SBUF-resident fp16 transposed activation accumulator (layer 1);
bias + log_softmax (layer 2).

Edge bookkeeping (sorting, slot assignment, padding so all 8 cores share
one instruction stream) is host-side numpy index work; all floating-point
math on features runs on device.
"""

import math

import numpy as np

import concourse.bass as bass
import concourse.mybir as mybir
import concourse.tile as tile
from concourse import library_config
from concourse.bass_utils import run_bass_kernel_spmd

N_NODES = 100000
N_EDGES = 1600000
IN_DIM, HID_DIM, OUT_DIM = 128, 64, 40
N_CORES = 8

RNG = 25000          # rows per gather range (must fit int16 indices)
NR = 4               # number of ranges covering N_NODES
CHUNK = 32           # tiles per dma_gather instruction
GBUFS = 10           # gather chunk buffers in flight
SBUFS = 8
PBUFS = 4

F32 = mybir.dt.float32
F16 = mybir.dt.float16
I16 = mybir.dt.int16


def _split_long_waits(nc, max_waits=1):
    """This toolchain's codegen rejects instructions carrying more than one
    semaphore wait; move extra waits onto preceding same-engine no-ops."""
    cnt = 0
    for bb in nc.main_func.blocks:
        i = 0
        insts = bb.instructions
        while i < len(insts):
            ins = insts[i]
            si = ins.sync_info
            if si is not None and si.on_wait and len(si.on_wait) > max_waits:
                waits = list(si.on_wait)
                keep = waits[-max_waits:]
                extra = waits[:-max_waits]
                si.on_wait = keep
                new_insts = []
                for j in range(0, len(extra), max_waits):
                    chunk = extra[j : j + max_waits]
                    nop = mybir.InstNoOp(
                        name=f"{ins.name}-waitsplit-{j}",
                        engine=ins.engine,
                        ins=[],
                        outs=[],
                        sync_info=mybir.SyncInfo(on_wait=chunk, on_update=[]),
                    )
                    new_insts.append(nop)
                insts[i:i] = new_insts
                i += len(new_insts)
                cnt += len(new_insts)
            i += 1
    return cnt


def _preprocess(edge_index, n_nodes, n_cores):
    """Host-side index bookkeeping. Returns per-core slot arrays + layout."""
    nloc = n_nodes // n_cores
    wn = math.ceil(nloc / 128)

    src = np.asarray(edge_index[0], dtype=np.int64)
    dst = np.asarray(edge_index[1], dtype=np.int64)
    loop = np.arange(n_nodes, dtype=np.int64)
    src_all = np.concatenate([src, loop])
    dst_all = np.concatenate([dst, loop])

    deg = np.bincount(dst_all, minlength=n_nodes).astype(np.float64)
    dis = np.where(deg > 0, 1.0 / np.sqrt(deg), 0.0)
    norm = (dis[src_all] * dis[dst_all]).astype(np.float32)

    core = dst_all // nloc
    dloc = dst_all - core * nloc
    w = dloc >> 7
    drel = (dloc & 127).astype(np.float32)
    r = src_all // RNG
    assert r.max() < NR

    key = (core * wn + w) * NR + r
    order = np.argsort(key, kind="stable")
    counts = np.bincount(key, minlength=n_cores * wn * NR).reshape(n_cores, wn, NR)

    # tiles per (window, range): shared across cores (single SPMD program)
    k_wr = np.maximum((counts + 127) // 128, 0).max(axis=0)  # [wn, NR]
    k_wr[:, 0] = np.maximum(k_wr[:, 0], 1)  # every window needs >=1 tile

    T_r = k_wr.sum(axis=0)  # tiles per range  [NR]
    t_start = np.zeros(NR + 1, dtype=np.int64)
    t_start[1:] = np.cumsum(T_r)
    t_total = int(t_start[NR])

    # global tile id of the first tile of (w, r): r-major layout
    trw = np.zeros((NR, wn), dtype=np.int64)
    for rr in range(NR):
        trw[rr, 0] = t_start[rr]
        trw[rr, 1:] = t_start[rr] + np.cumsum(k_wr[:, rr])[:-1]

    # rank of each edge within its (core, w, r) group, in sorted order
    grp_start = np.zeros(n_cores * wn * NR, dtype=np.int64)
    grp_start[1:] = np.cumsum(counts.reshape(-1))[:-1]
    rank = np.arange(len(order), dtype=np.int64) - grp_start[key[order]]

    w_o = w[order]
    r_o = r[order]
    slot = (trw[r_o, w_o] + (rank >> 7)) * 128 + (rank & 127)

    n_slots = t_total * 128
    idx_a = np.zeros((n_cores, n_slots), dtype=np.int16)
    drel_a = np.zeros((n_cores, n_slots), dtype=np.float32)
    nrm_a = np.zeros((n_cores, n_slots), dtype=np.float32)

    c_o = core[order]
    idx_a[c_o, slot] = (src_all[order] - r_o * RNG).astype(np.int16)
    drel_a[c_o, slot] = drel[order]
    nrm_a[c_o, slot] = norm[order]

    # device layouts:
    #   idx: [128, t_total*8] int16, slot s at [16g + s%16, s//16], g=0..7
    #   drel/nrm: [128, t_total] f32, slot s at [s%128, s//128]
    idx_dev, drel_dev, nrm_dev = [], [], []
    for c in range(n_cores):
        base = idx_a[c].reshape(n_slots // 16, 16).T  # [16, t_total*8]
        idx_dev.append(np.tile(base, (8, 1)).copy())
        drel_dev.append(drel_a[c].reshape(t_total, 128).T.copy())
        nrm_dev.append(nrm_a[c].reshape(t_total, 128).T.copy())

    return {
        "nloc": nloc,
        "wn": wn,
        "k_wr": k_wr,
        "T_r": [int(v) for v in T_r],
        "t_start": [int(v) for v in t_start],
        "trw": trw,
        "t_total": t_total,
        "idx": idx_dev,
        "drel": drel_dev,
        "norm": nrm_dev,
    }


def _build_nc(meta, n_nodes, hid, out_dim, in_dim, n_cores):
    nloc = meta["nloc"]
    wn = meta["wn"]
    k_wr = meta["k_wr"]
    T_r = meta["T_r"]
    t_start = meta["t_start"]
    trw = meta["trw"]
    t_total = meta["t_total"]

    nc = bass.Bass(num_devices=n_cores)

    xT16 = nc.dram_tensor("xT16", [in_dim, nloc], F16, kind="ExternalInput")
    idx = nc.dram_tensor("idx", [128, t_total * 8], I16, kind="ExternalInput")
    drel = nc.dram_tensor("drel", [128, t_total], F32, kind="ExternalInput")
    nrm = nc.dram_tensor("nrm", [128, t_total], F32, kind="ExternalInput")
    w1 = nc.dram_tensor("w1", [in_dim, hid], F16, kind="ExternalInput")
    w2p = nc.dram_tensor("w2p", [hid, hid], F16, kind="ExternalInput")
    b1c = nc.dram_tensor("b1c", [hid, 1], F32, kind="ExternalInput")
    b2rep = nc.dram_tensor("b2rep", [128, hid], F32, kind="ExternalInput")
    iota_in = nc.dram_tensor("iota", [128, 128], F16, kind="ExternalInput")
    out = nc.dram_tensor("out", [nloc, out_dim], F32, kind="ExternalOutput")

    nb = math.ceil(nloc / 128)
    eq = mybir.AluOpType.is_equal
    mul = mybir.AluOpType.mult

    with tile.TileContext(nc) as tc:
        with (
            tc.tile_pool(name="const", bufs=1) as cp,
            tc.tile_pool(name="gpool", bufs=GBUFS) as gp,
            tc.tile_pool(name="spool", bufs=SBUFS) as sp,
            tc.tile_pool(name="evac", bufs=4) as ep,
            tc.tile_pool(name="ps_agg", bufs=PBUFS, space="PSUM") as pa,
            tc.tile_pool(name="ps_mm", bufs=4, space="PSUM") as pm,
            tc.tile_pool(name="dram", bufs=1, space="DRAM") as dp,
        ):
            nc.gpsimd.load_library(library_config.mlp)

            # ---- resident tensors ----
            xT_t = cp.tile([in_dim, nloc], F16)
            nc.sync.dma_start(out=xT_t[:], in_=xT16[:])
            idx_t = cp.tile([128, t_total * 8], I16)
            nc.sync.dma_start(out=idx_t[:], in_=idx[:])
            drel_t = cp.tile([128, t_total], F32)
            nc.sync.dma_start(out=drel_t[:], in_=drel[:])
            nrm_t = cp.tile([128, t_total], F32)
            nc.sync.dma_start(out=nrm_t[:], in_=nrm[:])
            w1_t = cp.tile([in_dim, hid], F16)
            nc.sync.dma_start(out=w1_t[:], in_=w1[:])
            w2_t = cp.tile([hid, hid], F16)
            nc.sync.dma_start(out=w2_t[:], in_=w2p[:])
            b1_t = cp.tile([hid, 1], F32)
            nc.sync.dma_start(out=b1_t[:], in_=b1c[:])
            b2_t = cp.tile([128, hid], F32)
            nc.sync.dma_start(out=b2_t[:], in_=b2rep[:])
            iota_t = cp.tile([128, 128], F16)
            nc.sync.dma_start(out=iota_t[:], in_=iota_in[:])
            acc1T = cp.tile([hid, wn * 128], F16)

            h1loc = dp.tile([nloc, 2 * hid], F16)
            h1full = dp.tile([n_nodes, 2 * hid], F16, addr_space="Shared")
            h2loc = dp.tile([nloc, 2 * hid], F16)
            h2full = dp.tile([n_nodes, 2 * hid], F16, addr_space="Shared")

            def build_s(t):
                """S[e, j] = norm[e] * (dst_rel[e] == j), exact in f32."""
                s = sp.tile([128, 128], F16, tag="s", name="s")
                nc.vector.tensor_scalar(
                    out=s[:], in0=iota_t[:],
                    scalar1=drel_t[:, t : t + 1],
                    scalar2=nrm_t[:, t : t + 1],
                    op0=eq, op1=mul,
                )
                return s

            def pre_matmul(lhsT_t, w_t, dst_dram):
                """h = act_prev @ W for the local shard -> DRAM table."""
                for b in range(nb):
                    cols = min(128, nloc - b * 128)
                    ps = pm.tile([128, hid], F32, tag="pmm")
                    nc.tensor.matmul(
                        out=ps[:cols, :],
                        lhsT=lhsT_t[:, b * 128 : b * 128 + cols],
                        rhs=w_t[:],
                        start=True,
                        stop=True,
                    )
                    hb = ep.tile([128, hid], F16, tag="hb")
                    nc.vector.tensor_copy(out=hb[:cols, :], in_=ps[:cols, :])
                    nc.sync.dma_start(
                        out=dst_dram[b * 128 : b * 128 + cols, 0:hid],
                        in_=hb[:cols, :],
                    )

            def all_gather(src_dram, dst_dram):
                nc.gpsimd.collective_compute(
                    "AllGather",
                    mybir.AluOpType.bypass,
                    replica_groups=[list(range(n_cores))],
                    ins=[src_dram[:].opt()],
                    outs=[dst_dram[0:n_nodes, :].opt()],
                )

            nidx_regs = {}

            def nidx_reg(n):
                if n not in nidx_regs:
                    nidx_regs[n] = nc.gpsimd.to_reg(n)
                return nidx_regs[n]

            def agg_pass(table, layer):
                """Chunk-pipelined gather + per-window aggregation."""
                issued = [0] * NR          # tiles issued per range
                chunks = [[] for _ in range(NR)]  # (tile_ref, t0, span)

                def ensure(rr, tiles_needed):
                    while issued[rr] < tiles_needed:
                        t0 = t_start[rr] + issued[rr]
                        span = min(CHUNK, T_r[rr] - issued[rr])
                        g = gp.tile([128, CHUNK, 2 * hid], F16, tag="g", name="g")
                        rows0 = rr * RNG
                        rows1 = min(rows0 + RNG, n_nodes)
                        nc.gpsimd.dma_gather(
                            g[:, 0:span, :],
                            table[rows0:rows1, :],
                            idx_t[:, t0 * 8 : (t0 + span) * 8],
                            span * 128,
                            nidx_reg(span * 128),
                            2 * hid,
                            single_packet=False,
                        )
                        chunks[rr].append((g, t0, span))
                        issued[rr] += span

                def gview(rr, t):
                    while True:
                        g, t0, span = chunks[rr][0]
                        if t < t0 + span:
                            return g[:, t - t0, 0:hid]
                        chunks[rr].pop(0)

                for w in range(wn):
                    kw = [int(k_wr[w][rr]) for rr in range(NR)]
                    ktot = sum(kw)
                    for rr in range(NR):
                        if kw[rr]:
                            ensure(rr, trw[rr][w] - t_start[rr] + kw[rr])
                    if layer == 1:
                        pw = pa.tile([hid, 128], F32, tag="pagg")
                    else:
                        pw = pa.tile([128, hid], F32, tag="pagg")
                    ki = 0
                    for rr in range(NR):
                        for k in range(kw[rr]):
                            t = int(trw[rr][w]) + k
                            gv = gview(rr, t)
                            s = build_s(t)
                            if layer == 1:
                                nc.tensor.matmul(
                                    out=pw[:], lhsT=gv, rhs=s[:],
                                    start=(ki == 0), stop=(ki == ktot - 1),
                                )
                            else:
                                nc.tensor.matmul(
                                    out=pw[:], lhsT=s[:], rhs=gv,
                                    start=(ki == 0), stop=(ki == ktot - 1),
                                )
                            ki += 1
                    if layer == 1:
                        # epilogue: acc1T[:, w*128:] = relu(pw + b1)  (fp16)
                        nc.scalar.activation(
                            out=acc1T[:, w * 128 : (w + 1) * 128],
                            in_=pw[:],
                            func=mybir.ActivationFunctionType.Relu,
                            bias=b1_t[:],
                        )
                    else:
                        rows = min(128, nloc - w * 128)
                        zt = ep.tile([128, hid], F32, tag="zt")
                        nc.vector.tensor_tensor(
                            out=zt[:], in0=pw[:], in1=b2_t[:], op=mybir.AluOpType.add
                        )
                        mx = ep.tile([128, 1], F32, tag="mx")
                        nc.vector.reduce_max(
                            mx[:], zt[:, :out_dim], axis=mybir.AxisListType.X
                        )
                        sh = ep.tile([128, out_dim], F32, tag="sh")
                        nc.vector.tensor_scalar_sub(
                            out=sh[:], in0=zt[:, :out_dim], scalar1=mx[:]
                        )
                        ex = ep.tile([128, out_dim], F32, tag="ex")
                        sm = ep.tile([128, 1], F32, tag="sm")
                        nc.scalar.activation(
                            out=ex[:], in_=sh[:],
                            func=mybir.ActivationFunctionType.Exp,
                            accum_out=sm[:],
                        )
                        lnt = ep.tile([128, 1], F32, tag="lnt")
                        nc.scalar.activation(
                            out=lnt[:], in_=sm[:],
                            func=mybir.ActivationFunctionType.Ln,
                        )
                        res = ep.tile([128, out_dim], F32, tag="res")
                        nc.vector.tensor_scalar_sub(
                            out=res[:], in0=sh[:], scalar1=lnt[:]
                        )
                        nc.sync.dma_start(
                            out=out[w * 128 : w * 128 + rows, :], in_=res[:rows, :]
                        )

            # ---- pipeline ----
            pre_matmul(xT_t, w1_t, h1loc)
            all_gather(h1loc, h1full)
            agg_pass(h1full, layer=1)
            pre_matmul(acc1T, w2_t, h2loc)
            all_gather(h2loc, h2full)
            agg_pass(h2full, layer=2)

    _split_long_waits(nc)
    mybir.codegen_inst_isa_subclasses(nc)
    return nc


def _prepare(x, edge_index, W1, b1, W2, b2, n_nodes=N_NODES, n_cores=N_CORES):
    x = np.asarray(x, dtype=np.float32)
    W1 = np.asarray(W1, dtype=np.float32)
    b1 = np.asarray(b1, dtype=np.float32)
    W2 = np.asarray(W2, dtype=np.float32)
    b2 = np.asarray(b2, dtype=np.float32)

    in_dim = x.shape[1]
    hid = W1.shape[1]
    out_dim = W2.shape[1]

    meta = _preprocess(edge_index, n_nodes, n_cores)
    nloc = meta["nloc"]

    nc = _build_nc(meta, n_nodes, hid, out_dim, in_dim, n_cores)

    w2pad = np.zeros((hid, hid), dtype=np.float16)
    w2pad[:, :out_dim] = W2.astype(np.float16)
    b2p = np.zeros((hid,), dtype=np.float32)
    b2p[:out_dim] = b2
    b2rep = np.tile(b2p[None, :], (128, 1)).copy()
    b1c = b1.reshape(hid, 1).copy()
    iota = np.tile(np.arange(128, dtype=np.float16)[None, :], (128, 1)).copy()
    w1h = W1.astype(np.float16)

    in_maps = []
    for c in range(n_cores):
        xs = x[c * nloc : (c + 1) * nloc]
        in_maps.append(
            {
                "xT16": np.ascontiguousarray(xs.T.astype(np.float16)),
                "idx": meta["idx"][c],
                "drel": meta["drel"][c],
                "nrm": meta["norm"][c],
                "w1": w1h,
                "w2p": w2pad,
                "b1c": b1c,
                "b2rep": b2rep,
                "iota": iota,
            }
        )
    return nc, in_maps


def kernel(x, edge_index, W1, b1, W2, b2):
    nc, in_maps = _prepare(x, edge_index, W1, b1, W2, b2)
    res = run_bass_kernel_spmd(nc, in_maps, core_ids=list(range(N_CORES)))
    return np.concatenate([res.results[c]["out"] for c in range(N_CORES)], axis=0)


# revision 12
# speedup vs baseline: 5.3623x; 3.1211x over previous
"""Two-layer GCN (PyG GCNConv semantics) on 8 Trainium2 NeuronCores.

Strategy (sharding_hint): nodes are sharded row-wise across the 8 cores;
edges are partitioned by destination node so the segment-sum stays local;
source-node features are exchanged with an on-device AllGather between
layers; the small weight matrices are replicated.

Gather pipeline: per-edge source rows are fetched with batched SWDGE
dma_gather (int16 indices, thousands of rows per instruction) instead of
per-128-row indirect DMAs — this takes the Pool-engine descriptor
generation off the critical path.  Because dma_gather indices are int16,
the gather table is processed in 4 ranges of 32768 rows; edges are grouped
host-side by (destination window, source range) and padded to 128-slot
tiles so every tile is single-window and single-range.

Aggregation per 128-node destination window: a norm-weighted one-hot
S[e, j] = norm[e] * (dst_rel[e] == j) built in one DVE tensor_scalar per
tile, then PE matmuls accumulate msgs into PSUM.  Epilogues: relu+bias
into an SBUF-resident fp16 transposed activation accumulator (layer 1);
bias + log_softmax (layer 2).

Edge bookkeeping (sorting, slot assignment, padding so all 8 cores share
one instruction stream) is host-side numpy index work; all floating-point
math on features runs on device.
"""

import math

import numpy as np

import concourse.bass as bass
import concourse.mybir as mybir
import concourse.tile as tile
from concourse import library_config
from concourse.bass_utils import run_bass_kernel_spmd

N_NODES = 100000
N_EDGES = 1600000
IN_DIM, HID_DIM, OUT_DIM = 128, 64, 40
N_CORES = 8

RNG = 25000          # rows per gather range (must fit int16 indices)
NR = 4               # number of ranges covering N_NODES
CHUNK = 32           # tiles per dma_gather instruction
GBUFS = 10           # gather chunk buffers in flight
SBUFS = 24
PBUFS = 4

F32 = mybir.dt.float32
F16 = mybir.dt.float16
I16 = mybir.dt.int16


def _split_long_waits(nc, max_waits=1):
    """This toolchain's codegen rejects instructions carrying more than one
    semaphore wait; move extra waits onto preceding same-engine no-ops."""
    cnt = 0
    for bb in nc.main_func.blocks:
        i = 0
        insts = bb.instructions
        while i < len(insts):
            ins = insts[i]
            si = ins.sync_info
            if si is not None and si.on_wait and len(si.on_wait) > max_waits:
                waits = list(si.on_wait)
                keep = waits[-max_waits:]
                extra = waits[:-max_waits]
                si.on_wait = keep
                new_insts = []
                for j in range(0, len(extra), max_waits):
                    chunk = extra[j : j + max_waits]
                    nop = mybir.InstNoOp(
                        name=f"{ins.name}-waitsplit-{j}",
                        engine=ins.engine,
                        ins=[],
                        outs=[],
                        sync_info=mybir.SyncInfo(on_wait=chunk, on_update=[]),
                    )
                    new_insts.append(nop)
                insts[i:i] = new_insts
                i += len(new_insts)
                cnt += len(new_insts)
            i += 1
    return cnt


def _preprocess(edge_index, n_nodes, n_cores):
    """Host-side index bookkeeping. Returns per-core slot arrays + layout."""
    nloc = n_nodes // n_cores
    wn = math.ceil(nloc / 128)

    src = np.asarray(edge_index[0], dtype=np.int64)
    dst = np.asarray(edge_index[1], dtype=np.int64)
    loop = np.arange(n_nodes, dtype=np.int64)
    src_all = np.concatenate([src, loop])
    dst_all = np.concatenate([dst, loop])

    deg = np.bincount(dst_all, minlength=n_nodes).astype(np.float64)
    dis = np.where(deg > 0, 1.0 / np.sqrt(deg), 0.0)
    norm = (dis[src_all] * dis[dst_all]).astype(np.float32)

    core = dst_all // nloc
    dloc = dst_all - core * nloc
    w = dloc >> 7
    drel = (dloc & 127).astype(np.float32)
    r = src_all // RNG
    assert r.max() < NR

    key = (core * wn + w) * NR + r
    order = np.argsort(key, kind="stable")
    counts = np.bincount(key, minlength=n_cores * wn * NR).reshape(n_cores, wn, NR)

    # tiles per (window, range): shared across cores (single SPMD program)
    k_wr = np.maximum((counts + 127) // 128, 0).max(axis=0)  # [wn, NR]
    k_wr[:, 0] = np.maximum(k_wr[:, 0], 1)  # every window needs >=1 tile

    T_r = k_wr.sum(axis=0)  # tiles per range  [NR]
    t_start = np.zeros(NR + 1, dtype=np.int64)
    t_start[1:] = np.cumsum(T_r)
    t_total = int(t_start[NR])

    # global tile id of the first tile of (w, r): r-major layout
    trw = np.zeros((NR, wn), dtype=np.int64)
    for rr in range(NR):
        trw[rr, 0] = t_start[rr]
        trw[rr, 1:] = t_start[rr] + np.cumsum(k_wr[:, rr])[:-1]

    # rank of each edge within its (core, w, r) group, in sorted order
    grp_start = np.zeros(n_cores * wn * NR, dtype=np.int64)
    grp_start[1:] = np.cumsum(counts.reshape(-1))[:-1]
    rank = np.arange(len(order), dtype=np.int64) - grp_start[key[order]]

    w_o = w[order]
    r_o = r[order]
    slot = (trw[r_o, w_o] + (rank >> 7)) * 128 + (rank & 127)

    n_slots = t_total * 128
    idx_a = np.zeros((n_cores, n_slots), dtype=np.int16)
    drel_a = np.zeros((n_cores, n_slots), dtype=np.float32)
    nrm_a = np.zeros((n_cores, n_slots), dtype=np.float32)

    c_o = core[order]
    idx_a[c_o, slot] = (src_all[order] - r_o * RNG).astype(np.int16)
    drel_a[c_o, slot] = drel[order]
    nrm_a[c_o, slot] = norm[order]

    # device layouts:
    #   idx: [128, t_total*8] int16, slot s at [16g + s%16, s//16], g=0..7
    #   drel/nrm: [128, t_total] f32, slot s at [s%128, s//128]
    idx_dev, drel_dev, nrm_dev = [], [], []
    for c in range(n_cores):
        base = idx_a[c].reshape(n_slots // 16, 16).T  # [16, t_total*8]
        idx_dev.append(np.tile(base, (8, 1)).copy())
        drel_dev.append(drel_a[c].reshape(t_total, 128).T.copy())
        nrm_dev.append(nrm_a[c].reshape(t_total, 128).T.copy())

    return {
        "nloc": nloc,
        "wn": wn,
        "k_wr": k_wr,
        "T_r": [int(v) for v in T_r],
        "t_start": [int(v) for v in t_start],
        "trw": trw,
        "t_total": t_total,
        "idx": idx_dev,
        "drel": drel_dev,
        "norm": nrm_dev,
    }


def _build_nc(meta, n_nodes, hid, out_dim, in_dim, n_cores, rounds=1):
    nloc = meta["nloc"]
    wn = meta["wn"]
    k_wr = meta["k_wr"]
    T_r = meta["T_r"]
    t_start = meta["t_start"]
    trw = meta["trw"]
    t_total = meta["t_total"]

    nc = bass.Bass(num_devices=n_cores, num_swdge_queues=4)

    xT16 = nc.dram_tensor("xT16", [in_dim, nloc], F16, kind="ExternalInput")
    idx = nc.dram_tensor("idx", [128, t_total * 8], I16, kind="ExternalInput")
    drel = nc.dram_tensor("drel", [128, t_total], F32, kind="ExternalInput")
    nrm = nc.dram_tensor("nrm", [128, t_total], F32, kind="ExternalInput")
    nneg = nc.dram_tensor("nneg", [128, t_total], F32, kind="ExternalInput")
    w1 = nc.dram_tensor("w1", [in_dim, hid], F16, kind="ExternalInput")
    w2p = nc.dram_tensor("w2p", [hid, hid], F16, kind="ExternalInput")
    b1c = nc.dram_tensor("b1c", [hid, 1], F32, kind="ExternalInput")
    b2rep = nc.dram_tensor("b2rep", [128, hid], F32, kind="ExternalInput")
    iota_in = nc.dram_tensor("iota", [128, 128], F16, kind="ExternalInput")
    out = nc.dram_tensor("out", [nloc, out_dim], F32, kind="ExternalOutput")

    nb = math.ceil(nloc / 128)
    eq = mybir.AluOpType.is_equal
    mul = mybir.AluOpType.mult

    with tile.TileContext(nc) as tc:
        with (
            tc.tile_pool(name="const", bufs=1) as cp,
            tc.tile_pool(name="gpool", bufs=GBUFS) as gp,
            tc.tile_pool(name="spool", bufs=SBUFS) as sp,
            tc.tile_pool(name="evac", bufs=4) as ep,
            tc.tile_pool(name="ps_agg", bufs=PBUFS, space="PSUM") as pa,
            tc.tile_pool(name="ps_mm", bufs=4, space="PSUM") as pm,
            tc.tile_pool(name="dram", bufs=1, space="DRAM") as dp,
        ):
            nc.gpsimd.load_library(library_config.mlp)

            # ---- resident tensors ----
            xT_t = cp.tile([in_dim, nloc], F16)
            nc.sync.dma_start(out=xT_t[:], in_=xT16[:])
            idx_t = cp.tile([128, t_total * 8], I16)
            nc.sync.dma_start(out=idx_t[:], in_=idx[:])
            drel_t = cp.tile([128, t_total], F32)
            nc.sync.dma_start(out=drel_t[:], in_=drel[:])
            nrm_t = cp.tile([128, t_total], F32)
            nc.sync.dma_start(out=nrm_t[:], in_=nrm[:])
            nneg_t = cp.tile([128, t_total], F32)
            nc.sync.dma_start(out=nneg_t[:], in_=nneg[:])
            w1_t = cp.tile([in_dim, hid], F16)
            nc.sync.dma_start(out=w1_t[:], in_=w1[:])
            w2_t = cp.tile([hid, hid], F16)
            nc.sync.dma_start(out=w2_t[:], in_=w2p[:])
            b1_t = cp.tile([hid, 1], F32)
            nc.sync.dma_start(out=b1_t[:], in_=b1c[:])
            b2_t = cp.tile([128, hid], F32)
            nc.sync.dma_start(out=b2_t[:], in_=b2rep[:])
            iota_t = cp.tile([128, 128], F16)
            nc.sync.dma_start(out=iota_t[:], in_=iota_in[:])
            acc1T = cp.tile([hid, wn * 128], F16)

            h1loc = dp.tile([nloc, 2 * hid], F16)
            h2loc = dp.tile([nloc, 2 * hid], F16)

            def build_s(t):
                """S[e, j] = norm[e] * (dst_rel[e] == j)."""
                s = sp.tile([128, 128], F16, tag="s", name="s")
                if t % 6 == 5:
                    # ACT path: relu(norm - norm*(drel-iota)^2) == norm iff eq
                    tmp = sp.tile([128, 128], F16, tag="stmp", name="stmp")
                    nc.scalar.activation(
                        out=tmp[:], in_=iota_t[:],
                        func=mybir.ActivationFunctionType.Square,
                        bias=drel_t[:, t : t + 1], scale=-1.0,
                    )
                    nc.scalar.activation(
                        out=s[:], in_=tmp[:],
                        func=mybir.ActivationFunctionType.Relu,
                        bias=nrm_t[:, t : t + 1], scale=nneg_t[:, t : t + 1],
                    )
                else:
                    nc.vector.tensor_scalar(
                        out=s[:], in0=iota_t[:],
                        scalar1=drel_t[:, t : t + 1],
                        scalar2=nrm_t[:, t : t + 1],
                        op0=eq, op1=mul,
                    )
                return s

            def pre_matmul(lhsT_t, w_t, dst_dram):
                """h = act_prev @ W for the local shard -> DRAM table."""
                for b in range(nb):
                    cols = min(128, nloc - b * 128)
                    ps = pm.tile([128, hid], F32, tag="pmm")
                    nc.tensor.matmul(
                        out=ps[:cols, :],
                        lhsT=lhsT_t[:, b * 128 : b * 128 + cols],
                        rhs=w_t[:],
                        start=True,
                        stop=True,
                    )
                    hb = ep.tile([128, hid], F16, tag="hb")
                    nc.vector.tensor_copy(out=hb[:cols, :], in_=ps[:cols, :])
                    nc.sync.dma_start(
                        out=dst_dram[b * 128 : b * 128 + cols, 0:hid],
                        in_=hb[:cols, :],
                    )

            def all_gather(src_dram, dst_dram):
                nc.gpsimd.collective_compute(
                    "AllGather",
                    mybir.AluOpType.bypass,
                    replica_groups=[list(range(n_cores))],
                    ins=[src_dram[:].opt()],
                    outs=[dst_dram[0:n_nodes, :].opt()],
                )

            nidx_regs = {}

            def nidx_reg(n):
                if n not in nidx_regs:
                    nidx_regs[n] = nc.gpsimd.to_reg(n)
                return nidx_regs[n]

            def agg_pass(table, layer):
                """Chunk-pipelined gather + per-window aggregation."""
                issued = [0] * NR          # tiles issued per range
                chunks = [[] for _ in range(NR)]  # (tile_ref, t0, span)

                def ensure(rr, tiles_needed):
                    while issued[rr] < tiles_needed:
                        t0 = t_start[rr] + issued[rr]
                        span = min(CHUNK, T_r[rr] - issued[rr])
                        g = gp.tile([128, CHUNK, 2 * hid], F16, tag="g", name="g")
                        rows0 = rr * RNG
                        rows1 = min(rows0 + RNG, n_nodes)
                        nc.gpsimd.dma_gather(
                            g[:, 0:span, :],
                            table[rows0:rows1, :],
                            idx_t[:, t0 * 8 : (t0 + span) * 8],
                            span * 128,
                            nidx_reg(span * 128),
                            2 * hid,
                            single_packet=False,
                            queue_num=rr,
                        )
                        chunks[rr].append((g, t0, span))
                        issued[rr] += span

                def gview(rr, t):
                    while True:
                        g, t0, span = chunks[rr][0]
                        if t < t0 + span:
                            return g[:, t - t0, 0:hid]
                        chunks[rr].pop(0)

                for w in range(wn):
                    kw = [int(k_wr[w][rr]) for rr in range(NR)]
                    ktot = sum(kw)
                    for rr in range(NR):
                        if kw[rr]:
                            ensure(rr, trw[rr][w] - t_start[rr] + kw[rr])
                    if layer == 1:
                        pw = pa.tile([hid, 128], F32, tag="pagg")
                    else:
                        pw = pa.tile([128, hid], F32, tag="pagg")
                    ki = 0
                    for rr in range(NR):
                        for k in range(kw[rr]):
                            t = int(trw[rr][w]) + k
                            gv = gview(rr, t)
                            s = build_s(t)
                            if layer == 1:
                                nc.tensor.matmul(
                                    out=pw[:], lhsT=gv, rhs=s[:],
                                    start=(ki == 0), stop=(ki == ktot - 1),
                                )
                            else:
                                nc.tensor.matmul(
                                    out=pw[:], lhsT=s[:], rhs=gv,
                                    start=(ki == 0), stop=(ki == ktot - 1),
                                )
                            ki += 1
                    if layer == 1:
                        # epilogue: acc1T[:, w*128:] = relu(pw + b1)  (fp16)
                        nc.scalar.activation(
                            out=acc1T[:, w * 128 : (w + 1) * 128],
                            in_=pw[:],
                            func=mybir.ActivationFunctionType.Relu,
                            bias=b1_t[:],
                        )
                    else:
                        rows = min(128, nloc - w * 128)
                        zt = ep.tile([128, hid], F32, tag="zt")
                        nc.vector.tensor_tensor(
                            out=zt[:], in0=pw[:], in1=b2_t[:], op=mybir.AluOpType.add
                        )
                        mx = ep.tile([128, 1], F32, tag="mx")
                        nc.vector.reduce_max(
                            mx[:], zt[:, :out_dim], axis=mybir.AxisListType.X
                        )
                        sh = ep.tile([128, out_dim], F32, tag="sh")
                        nc.vector.tensor_scalar_sub(
                            out=sh[:], in0=zt[:, :out_dim], scalar1=mx[:]
                        )
                        ex = ep.tile([128, out_dim], F32, tag="ex")
                        sm = ep.tile([128, 1], F32, tag="sm")
                        nc.scalar.activation(
                            out=ex[:], in_=sh[:],
                            func=mybir.ActivationFunctionType.Exp,
                            accum_out=sm[:],
                        )
                        lnt = ep.tile([128, 1], F32, tag="lnt")
                        nc.scalar.activation(
                            out=lnt[:], in_=sm[:],
                            func=mybir.ActivationFunctionType.Ln,
                        )
                        res = ep.tile([128, out_dim], F32, tag="res")
                        nc.vector.tensor_scalar_sub(
                            out=res[:], in0=sh[:], scalar1=lnt[:]
                        )
                        nc.sync.dma_start(
                            out=out[w * 128 : w * 128 + rows, :], in_=res[:rows, :]
                        )

            # ---- pipeline ----
            for rnd in range(rounds):
                h1full = dp.tile(
                    [n_nodes, 2 * hid], F16, addr_space="Shared",
                    tag=f"h1full{rnd}", name=f"h1full{rnd}",
                )
                h2full = dp.tile(
                    [n_nodes, 2 * hid], F16, addr_space="Shared",
                    tag=f"h2full{rnd}", name=f"h2full{rnd}",
                )
                pre_matmul(xT_t, w1_t, h1loc)
                all_gather(h1loc, h1full)
                agg_pass(h1full, layer=1)
                pre_matmul(acc1T, w2_t, h2loc)
                all_gather(h2loc, h2full)
                agg_pass(h2full, layer=2)

    _split_long_waits(nc)
    mybir.codegen_inst_isa_subclasses(nc)
    return nc


def _prepare(x, edge_index, W1, b1, W2, b2, n_nodes=N_NODES, n_cores=N_CORES):
    x = np.asarray(x, dtype=np.float32)
    W1 = np.asarray(W1, dtype=np.float32)
    b1 = np.asarray(b1, dtype=np.float32)
    W2 = np.asarray(W2, dtype=np.float32)
    b2 = np.asarray(b2, dtype=np.float32)

    in_dim = x.shape[1]
    hid = W1.shape[1]
    out_dim = W2.shape[1]

    meta = _preprocess(edge_index, n_nodes, n_cores)
    nloc = meta["nloc"]

    nc = _build_nc(meta, n_nodes, hid, out_dim, in_dim, n_cores)

    w2pad = np.zeros((hid, hid), dtype=np.float16)
    w2pad[:, :out_dim] = W2.astype(np.float16)
    b2p = np.zeros((hid,), dtype=np.float32)
    b2p[:out_dim] = b2
    b2rep = np.tile(b2p[None, :], (128, 1)).copy()
    b1c = b1.reshape(hid, 1).copy()
    iota = np.tile(np.arange(128, dtype=np.float16)[None, :], (128, 1)).copy()
    w1h = W1.astype(np.float16)

    in_maps = []
    for c in range(n_cores):
        xs = x[c * nloc : (c + 1) * nloc]
        in_maps.append(
            {
                "xT16": np.ascontiguousarray(xs.T.astype(np.float16)),
                "idx": meta["idx"][c],
                "drel": meta["drel"][c],
                "nrm": meta["norm"][c],
                "nneg": -meta["norm"][c],
                "w1": w1h,
                "w2p": w2pad,
                "b1c": b1c,
                "b2rep": b2rep,
                "iota": iota,
            }
        )
    return nc, in_maps


def kernel(x, edge_index, W1, b1, W2, b2):
    nc, in_maps = _prepare(x, edge_index, W1, b1, W2, b2)
    res = run_bass_kernel_spmd(nc, in_maps, core_ids=list(range(N_CORES)))
    return np.concatenate([res.results[c]["out"] for c in range(N_CORES)], axis=0)
